# revision 11
# baseline (speedup 1.0000x reference)
"""Trainium2 Bass kernel: SMPL forward kinematics (6D pose -> global 6D rots).

Math (per frame, per joint): u = d6[0:3], a = d6[3:6]
  c1 = u x a               (cross)
  v  = c1 x u  = (u.u) a - (u.a) u
  d11 = u.u;  s = c1.c1;  d22 = v.v = s*d11  (c1 perp u)
  r1 = rsqrt(d11+eps), t = rsqrt(s+eps), r2 = rsqrt(d22+eps)
  R_local rows = [b1; b2; b3] = [r1*u; r2*v; t*c1]
FK: rows 0,1 of R_g[j] = (rows 0,1 of R_g[parent]) @ R_local[j]  -- row 2 of
R_g is never needed (child rows 0,1 only read parent rows 0,1), so it is
never computed.  Output = rows 0,1 of every R_g.

Layout: pure data parallel over frames: N = B*T = 8 cores x 128 partitions
x 98 frames.  Within a partition frames live in the free dim, processed in
2 chunks of F=49.  All tensors are PLANAR fp16: plane q = one channel x 49
contiguous frames, so every vector op has a unit-stride fp16 innermost dim
(DVE 2x mode).  The host pre-transposes/converts (not counted in device
time).

Engines: DVE carries the main chain; GPSIMD (plain tensor_tensor) takes
per-chunk side work; ScalarE does squares + rsqrt (Ln/Exp) + root copy.
"""

import numpy as np

import concourse.bass as bass
import concourse.bacc as bacc
import concourse.tile as tile
import concourse.mybir as mybir
from concourse.bass_utils import run_bass_kernel_spmd

P = 128
NCORES = 8
J = 22
F = 49            # frames per chunk (per partition)
NCHUNKS = 2
EPS = 1e-7

_compiled_cache = {}


def _levels_runs(parent):
    """BFS levels of the tree; each level a list of runs (j0, nj, p0, ps)
    with consecutive child joints (stride 1) and parent stride ps in {0,1}."""
    parent = [int(x) for x in parent]
    depth = [0] * J
    for j in range(1, J):
        depth[j] = depth[parent[j]] + 1
    levels = []
    for d in range(1, max(depth) + 1):
        joints = sorted(j for j in range(J) if depth[j] == d)
        runs = []
        i = 0
        while i < len(joints):
            j0, p0 = joints[i], parent[joints[i]]
            n = 1
            ps = None
            while i + n < len(joints):
                jn = joints[i + n]
                if jn != joints[i + n - 1] + 1:
                    break
                dps = parent[jn] - parent[joints[i + n - 1]]
                if dps not in (0, 1):
                    break
                if ps is None:
                    ps = dps
                elif dps != ps:
                    break
                n += 1
            if ps is None:
                ps = 1
            runs.append((j0, n, p0, ps))
            i += n
        levels.append(runs)
    return levels


# per-chunk engine assignment: chunk 0 keeps the fill path on DVE, later
# chunks push front-of-chain work to GPSIMD so it runs under chunk k-1's
# DVE FK.
def default_assign(ch):
    if ch == 0:
        return {"c1": "D", "vt": "D", "dadds": "D", "b1": "D", "b2": "D",
                "b3": "D", "fk": "D"}
    return {"c1": "P", "vt": "D", "dadds": "P", "b1": "D", "b2": "D",
            "b3": "D", "fk": "D"}


def _build(parent, repeat=1, assign_fn=None):
    """x: [P, NCHUNKS*132*F] fp16 planar -> y same shape fp16 planar."""
    if assign_fn is None:
        assign_fn = default_assign
    JF = J * F
    CF = 132 * F          # per-chunk per-partition elems (in and out)
    nc = bacc.Bacc("TRN2", debug=False)
    f16 = mybir.dt.float16
    x = nc.dram_tensor("x", [P, NCHUNKS * CF], f16, kind="ExternalInput")
    y = nc.dram_tensor("y", [P, NCHUNKS * CF], f16, kind="ExternalOutput")

    levels = _levels_runs(parent)
    AF = mybir.ActivationFunctionType

    # register EPS as a const AP so activation(bias=EPS) can use it
    epst = nc.alloc_sbuf_tensor("const-eps", [P, 1], mybir.dt.float32)
    nc.gpsimd.memset(epst.ap(), EPS)
    nc.const_aps.aps[(mybir.dt.float32, EPS)] = epst.ap()
    nc.all_engine_barrier()

    def ap(t, off, dims):
        return bass.AP(
            tensor=t.tensor,
            offset=t.offset + off,
            ap=[list(t.ap[0])] + [[s, n] for s, n in dims],
        )

    def mul(which, out, a, b):
        (nc.vector if which == "D" else nc.gpsimd).tensor_mul(out, a, b)

    def sub(which, out, a, b):
        (nc.vector if which == "D" else nc.gpsimd).tensor_sub(out, a, b)

    def add(which, out, a, b):
        (nc.vector if which == "D" else nc.gpsimd).tensor_add(out, a, b)

    from contextlib import ExitStack
    with tile.TileContext(nc) as tc:
        with (
            tc.tile_pool(name="io", bufs=2) as io_pool,
            tc.tile_pool(name="sc", bufs=2) as sc_pool,
            ExitStack() as stack,
        ):
            if repeat > 1:
                stack.enter_context(tc.For_i(0, repeat, 1))
            for ch in range(NCHUNKS):
                asg = assign_fn(ch)
                xin = io_pool.tile([P, CF], f16, tag="xin")
                nc.sync.dma_start(out=xin, in_=x[:, ch * CF:(ch + 1) * CF])
                yout = io_pool.tile([P, 6 * JF], f16, tag="yout")
                c1 = sc_pool.tile([P, 3 * JF], f16, tag="c1")
                vt = sc_pool.tile([P, 3 * JF], f16, tag="vt")
                sq = sc_pool.tile([P, 3 * JF], f16, tag="sq")   # u^2 planes
                scp = sc_pool.tile([P, 3 * JF], f16, tag="scp")  # c1^2 planes
                dots = sc_pool.tile([P, 6 * JF], f16, tag="dots")
                Rl = sc_pool.tile([P, 9 * JF], f16, tag="Rl")
                scr = sc_pool.tile([P, 6 * JF], f16, tag="scr")

                def pl(t, q, n=1):
                    if n == 1:
                        return ap(t, q * JF, [(1, JF)])
                    return ap(t, q * JF, [(JF, n), (1, JF)])

                # dots planes: 0=d11, 1=s, 2=d22 | 3=r1, 4=t, 5=r2
                # --- squares of u (ScalarE) ---
                nc.scalar.activation(ap(sq, 0, [(1, 3 * JF)]),
                                     ap(xin, 0, [(1, 3 * JF)]), AF.Square)
                # --- c1 = u x a2 ---
                w = asg["c1"]
                for (e, pa, qa, pb, qb) in ((0, 1, 5, 2, 4),
                                            (1, 2, 3, 0, 5),
                                            (2, 0, 4, 1, 3)):
                    mul(w, pl(c1, e), pl(xin, pa), pl(xin, qa))
                    mul(w, pl(scr, e), pl(xin, pb), pl(xin, qb))
                    sub(w, pl(c1, e), pl(c1, e), pl(scr, e))
                # --- squares of c1 (ScalarE) ---
                nc.scalar.activation(ap(scp, 0, [(1, 3 * JF)]),
                                     ap(c1, 0, [(1, 3 * JF)]), AF.Square)
                # --- d11 = sum sq, s = sum scp, d22 = s*d11 ---
                w = asg["dadds"]
                add(w, pl(dots, 0), pl(sq, 0), pl(sq, 1))
                add(w, pl(dots, 0), pl(dots, 0), pl(sq, 2))
                add(w, pl(dots, 1), pl(scp, 0), pl(scp, 1))
                add(w, pl(dots, 1), pl(dots, 1), pl(scp, 2))
                mul(w, pl(dots, 2), pl(dots, 0), pl(dots, 1))
                # --- rsqrt via Ln/Exp (ScalarE): r1 first, then (t, r2) ---
                nc.scalar.activation(ap(dots, 3 * JF, [(1, JF)]),
                                     ap(dots, 0, [(1, JF)]), AF.Ln, bias=EPS)
                nc.scalar.activation(ap(dots, 3 * JF, [(1, JF)]),
                                     ap(dots, 3 * JF, [(1, JF)]), AF.Exp,
                                     scale=-0.5)
                nc.scalar.activation(ap(dots, 4 * JF, [(1, 2 * JF)]),
                                     ap(dots, JF, [(1, 2 * JF)]), AF.Ln, bias=EPS)
                nc.scalar.activation(ap(dots, 4 * JF, [(1, 2 * JF)]),
                                     ap(dots, 4 * JF, [(1, 2 * JF)]), AF.Exp,
                                     scale=-0.5)
                r1b = ap(dots, 3 * JF, [(0, 3), (1, JF)])
                ttb = ap(dots, 4 * JF, [(0, 3), (1, JF)])
                r2b = ap(dots, 5 * JF, [(0, 3), (1, JF)])
                # --- vt = c1 x u (fills DVE during ScalarE rsqrt) ---
                w = asg["vt"]
                for (e, pa, qa, pb, qb) in ((0, 1, 2, 2, 1),
                                            (1, 2, 0, 0, 2),
                                            (2, 0, 1, 1, 0)):
                    mul(w, pl(vt, e), pl(c1, pa), pl(xin, qa))
                    mul(w, pl(scr, 3 + e), pl(c1, pb), pl(xin, qb))
                    sub(w, pl(vt, e), pl(vt, e), pl(scr, 3 + e))
                # --- R_local rows: b1 = r1*u, b3 = t*c1, b2 = r2*vt ---
                mul(asg["b1"], pl(Rl, 0, 3), ap(xin, 0, [(JF, 3), (1, JF)]), r1b)
                mul(asg["b3"], pl(Rl, 6, 3), ap(c1, 0, [(JF, 3), (1, JF)]), ttb)
                mul(asg["b2"], pl(Rl, 3, 3), ap(vt, 0, [(JF, 3), (1, JF)]), r2b)

                # --- root output rows = b1, b2 (joint 0 slice) ---
                nc.scalar.activation(
                    ap(yout, 0, [(3 * JF, 2), (JF, 3), (1, F)]),
                    ap(Rl, 0, [(3 * JF, 2), (JF, 3), (1, F)]), AF.Copy)

                # --- FK by level ---
                w = asg["fk"]
                for runs in levels:
                    for (j0, nj, p0, ps) in runs:
                        njF = nj * F
                        mkA = sc_pool.tile([P, 6 * njF], f16, tag=f"mkA{nj}")
                        mkB = sc_pool.tile([P, 6 * njF], f16, tag=f"mkB{nj}")
                        out_run = ap(yout, j0 * F, [(3 * JF, 2), (JF, 3), (1, njF)])
                        flatA = ap(mkA, 0, [(1, 6 * njF)])
                        flatB = ap(mkB, 0, [(1, 6 * njF)])
                        if ps == 1:
                            Ak = lambda k: ap(yout, k * JF + p0 * F,
                                              [(3 * JF, 2), (0, 3), (1, njF)])
                            Bk = lambda k: ap(Rl, k * 3 * JF + j0 * F,
                                              [(0, 2), (JF, 3), (1, njF)])
                            mka = ap(mkA, 0, [(3 * njF, 2), (njF, 3), (1, njF)])
                            mkb = ap(mkB, 0, [(3 * njF, 2), (njF, 3), (1, njF)])
                            mul(w, mka, Ak(0), Bk(0))
                            mul(w, mkb, Ak(1), Bk(1))
                            add(w, flatA, flatA, flatB)
                            mul(w, mkb, Ak(2), Bk(2))
                            add(w, out_run, flatA, flatB)
                        else:
                            for k in range(3):
                                Ak = ap(yout, k * JF + p0 * F,
                                        [(3 * JF, 2), (0, nj), (1, F)])
                                for i in range(3):
                                    Bki = ap(Rl, (k * 3 + i) * JF + j0 * F,
                                             [(0, 2), (F, nj), (1, F)])
                                    dst = mkA if k == 0 else mkB
                                    di = ap(dst, i * njF,
                                            [(3 * njF, 2), (F, nj), (1, F)])
                                    mul(w, di, Ak, Bki)
                                if k == 1:
                                    add(w, flatA, flatA, flatB)
                            add(w, out_run, flatA, flatB)

                nc.sync.dma_start(out=y[:, ch * CF:(ch + 1) * CF], in_=yout)
    nc.compile()
    return nc


# host-side channel permutations
_PERM_IN = np.array([(q % J) * 6 + q // J for q in range(132)])     # plane q <- chan
_PERM_OUT = np.array([(c % 6) * J + c // 6 for c in range(132)])    # chan c <- plane


def _run(pred_pose, parent, assign_fn=None):
    pred_pose = np.asarray(pred_pose, dtype=np.float32)
    parent = np.asarray(parent)
    B, T, C = pred_pose.shape
    N = B * T
    per_core = N // NCORES
    fpp = per_core // P            # 98
    assert fpp == NCHUNKS * F and C == 132

    key = tuple(int(p) for p in parent)
    if assign_fn is None and key in _compiled_cache:
        nc = _compiled_cache[key]
    else:
        nc = _build(parent, assign_fn=assign_fn)
        if assign_fn is None:
            _compiled_cache[key] = nc

    flat = pred_pose.reshape(N, C).astype(np.float16)
    in_maps = []
    for c in range(NCORES):
        blk = flat[c * per_core:(c + 1) * per_core].reshape(P, NCHUNKS, F, C)
        xdev = np.ascontiguousarray(
            blk[:, :, :, _PERM_IN].transpose(0, 1, 3, 2)).reshape(P, -1)
        in_maps.append({"x": xdev})
    res = run_bass_kernel_spmd(nc, in_maps, core_ids=list(range(NCORES)))
    out = np.empty((N, C), dtype=np.float32)
    for c in range(NCORES):
        ydev = np.asarray(res.results[c]["y"]).reshape(P, NCHUNKS, 132, F)
        blk = ydev.transpose(0, 1, 3, 2)[:, :, :, _PERM_OUT]
        out[c * per_core:(c + 1) * per_core] = \
            blk.reshape(per_core, C).astype(np.float32)
    return out.reshape(B, T, C), res


def kernel(pred_pose, parent):
    out, _ = _run(pred_pose, parent)
    return out
